# revision 37
# baseline (speedup 1.0000x reference)
"""Trainium2 Bass kernel for nn_Conv3DSynthesisLayer.

Computes, per sample b (one NeuronCore each, data-parallel over batch B=8):
  styles = w[b] @ (affine_weight / sqrt(512)).T + affine_bias        [Cin]
  wmod   = weight * styles[None,:,None..] ; demod by rsqrt(sumsq)    [Cout,Cin,3,3,3]
  out    = lrelu(conv3d(x[b], wmod, pad=1) + bias) * sqrt(2)         [Cout,32,32,32]

v3 (Winograd, ~330us vs 434us direct-bf16 baseline):
  Winograd F(2,3) along the W axis, bf16 (hw rel err 3.1e-3).
  - The kw-dim of the conv runs in the 4-point transform domain: per
    2-output tile t (16 tiles/row, padded coords), X~0 = x[2t-1]-x[2t+1],
    X~1 = x[2t]+x[2t+1], X~2 = x[2t+1]-x[2t], X~3 = x[2t]-x[2t+2];
    W~0 = w0, W~1 = (w0+w1+w2)/2, W~2 = (w0-w1+w2)/2, W~3 = w2.
  - Per output d-slice: 4 psum banks (one per j), each accumulating 9
    (kd,kh)-tap matmuls of full 512 rows (K=Cin=128 on partitions):
    36 matmuls/slice vs 54 for direct conv -> 1.5x less PE work.
    (fp8 e4m3 DoubleRow would be 2x but fails the 2e-2 gate at 3.6e-2;
    int8 and e3m4 perf modes are rejected by walrus/the TRN2 ISA.)
  - Phase A ordering keeps the in-order PE queue clear for an early conv
    start: GpSimd does the G-matrix combos in natural [co,ci] layout
    straight from wnat; the wjv transposes feed V (tensor_scalar_mul) and
    ScalarE (Identity-with-scale) directly from PSUM; the demod-sumsq
    squares+matmuls are emitted after the wjv path.
  - Inverse transform (even w: m0+m1+m2, odd w: m1-m2-m3): ScalarE drains
    m1/m2 to SBUF (TensorTensor allows one PSUM operand; GpSimd cannot
    touch PSUM), V adds with ps0/ps3, G the SBUF-only ops, then the fused
    Prelu epilogue (demod scale + bias + lrelu gain) on ScalarE with
    strided even/odd writes, then DMA per slice.
  - X~ is built from the raw staged x in half-block chunks trickled at the
    top of d-iterations, so V/G queue bursts stay ~2us and never delay the
    PSUM-releasing epilogue ops.
"""
import sys

sys.path.insert(0, "/opt/trn_rl_repo")

import numpy as np
from contextlib import ExitStack

import concourse.mybir as mybir
import concourse.tile as tile
from concourse import bacc
from concourse.masks import make_identity
from concourse.bass_utils import run_bass_kernel_spmd

F32 = mybir.dt.float32
BF16 = mybir.dt.bfloat16
AF = mybir.ActivationFunctionType

B, CIN, COUT, R = 8, 128, 128, 32
W_DIM = 512
NTAPS = 27
HP = R + 2    # 34: padded h extent of X~
NJ = 4        # winograd transform points
NT = 16       # w-tiles per row (2 outputs each)
GAIN = float(np.sqrt(2.0).astype(np.float32))
SLOPE = 0.2
EPS = 1e-8
DBLK = 4      # d-slices per x block
NBLK = R // DBLK
NCORES = 8

_cache = {}


def _build():
    nc = bacc.Bacc("TRN2", target_bir_lowering=False, debug=False, num_devices=NCORES)
    x_d = nc.dram_tensor("x", [CIN, R * R * R], F32, kind="ExternalInput").ap()
    wv_d = nc.dram_tensor("wvec", [W_DIM], F32, kind="ExternalInput").ap()
    wt_d = nc.dram_tensor("weight", [COUT, CIN * NTAPS], F32, kind="ExternalInput").ap()
    aw_d = nc.dram_tensor("aw", [CIN, W_DIM], F32, kind="ExternalInput").ap()
    ab_d = nc.dram_tensor("ab", [CIN], F32, kind="ExternalInput").ap()
    bs_d = nc.dram_tensor("bias", [COUT], F32, kind="ExternalInput").ap()
    out_d = nc.dram_tensor("out", [COUT, R * R * R], F32, kind="ExternalOutput").ap()

    ctx = ExitStack()
    with ctx:
        tc = ctx.enter_context(tile.TileContext(nc))
        singles = ctx.enter_context(tc.tile_pool(name="singles", bufs=1))
        xpool = ctx.enter_context(tc.tile_pool(name="xpool", bufs=4))
        stpool = ctx.enter_context(tc.tile_pool(name="stpool", bufs=3))
        sqpool = ctx.enter_context(tc.tile_pool(name="sqpool", bufs=3))
        upool = ctx.enter_context(tc.tile_pool(name="upool", bufs=9))
        tpool = ctx.enter_context(tc.tile_pool(name="tpool", bufs=2))
        obpool = ctx.enter_context(tc.tile_pool(name="obpool", bufs=3))

        # ---- param DMAs up front ----
        aw_sb = singles.tile([128, W_DIM], F32)
        nc.sync.dma_start(out=aw_sb, in_=aw_d)
        wnat = singles.tile([128, CIN * NTAPS], F32)
        nc.sync.dma_start(out=wnat, in_=wt_d)
        wv_sb = singles.tile([128, 4], F32)
        nc.sync.dma_start(out=wv_sb, in_=wv_d.rearrange("(c k) -> k c", k=128))
        ab_sb = singles.tile([128, 1], F32)
        nc.sync.dma_start(out=ab_sb, in_=ab_d.rearrange("(p one) -> p one", one=1))
        bs_sb = singles.tile([128, 1], F32)
        nc.sync.dma_start(out=bs_sb, in_=bs_d.rearrange("(p one) -> p one", one=1))

        # ---- x staging + winograd input transform ----
        x_r = x_d.rearrange("p (d hw) -> p d hw", hw=R * R)
        xblocks = [None] * NBLK

        def stage_block(blk):
            stag = stpool.tile([128, DBLK, R, R], F32, tag="stag", name=f"st{blk}")
            nc.sync.dma_start(
                out=stag.rearrange("p d h w -> p d (h w)"),
                in_=x_r[:, blk * DBLK:(blk + 1) * DBLK, :])
            return stag

        def transform_part(blk, stag, s0, s1):
            """X~[si, hp(34), j(4), t(16)] bf16 from raw stag, si in [s0,s1)."""
            if xblocks[blk] is None:
                xblocks[blk] = xpool.tile([128, DBLK, HP, NJ, NT], BF16,
                                          tag="xt", name=f"xt{blk}")
            xt = xblocks[blk]
            v = nc.vector
            g = nc.gpsimd
            st = stag[:, s0:s1]
            # h-pad rows 0 and 33 are transforms of zero rows
            v.memset(xt[:, s0:s1, 0, :, :], 0.0)
            g.memset(xt[:, s0:s1, HP - 1, :, :], 0.0)
            hp = xt[:, s0:s1, 1:R + 1, :, :]
            # j0 = x[2t-1] - x[2t+1]: t=0 -> -x[1]; t 1..15
            v.tensor_scalar_mul(out=hp[:, :, :, 0, 0:1],
                                in0=st[:, :, :, 1:2], scalar1=-1.0)
            v.tensor_sub(out=hp[:, :, :, 0, 1:16],
                         in0=st[:, :, :, 1:30:2], in1=st[:, :, :, 3:32:2])
            # j1 = x[2t] + x[2t+1];  j2 = x[2t+1] - x[2t]
            v.tensor_add(out=hp[:, :, :, 1, :],
                         in0=st[:, :, :, 0:32:2], in1=st[:, :, :, 1:32:2])
            g.tensor_sub(out=hp[:, :, :, 2, :],
                         in0=st[:, :, :, 1:32:2], in1=st[:, :, :, 0:32:2])
            # j3 = x[2t] - x[2t+2]: t 0..14; t=15 -> x[30]
            g.tensor_sub(out=hp[:, :, :, 3, 0:15],
                         in0=st[:, :, :, 0:29:2], in1=st[:, :, :, 2:31:2])
            g.tensor_copy(out=hp[:, :, :, 3, 15:16], in_=st[:, :, :, 30:31])

        def transform_block(blk, stag):
            transform_part(blk, stag, 0, DBLK)

        st0 = stage_block(0)
        st1 = stage_block(1)
        st2 = stage_block(2)

        # ---- phase A: styles, winograd-domain bf16 weights, demod scale ----
        with tc.tile_pool(name="ps_a", bufs=2, space="PSUM") as ps_a:
            ident = singles.tile([128, 128], F32)
            make_identity(nc, ident)

            # affine_weight.T, chunked over the 512-dim
            awt = singles.tile([128, W_DIM], F32)
            for c in range(4):
                paw = ps_a.tile([128, 128], F32, tag="paw", name=f"paw{c}")
                nc.tensor.transpose(paw, aw_sb[:, c * 128:(c + 1) * 128], ident)
                nc.vector.tensor_copy(out=awt[:, c * 128:(c + 1) * 128], in_=paw)

            ps_sty = ps_a.tile([128, 1], F32, tag="ps_sty")
            for c in range(4):
                nc.tensor.matmul(ps_sty, lhsT=awt[:, c * 128:(c + 1) * 128],
                                 rhs=wv_sb[:, c:c + 1], start=(c == 0), stop=(c == 3))
            styles = singles.tile([128, 1], F32)
            nc.scalar.activation(out=styles, in_=ps_sty, func=AF.Identity,
                                 bias=ab_sb, scale=1.0 / float(np.sqrt(W_DIM)))
            styles2 = singles.tile([128, 1], F32)
            nc.vector.tensor_mul(out=styles2, in0=styles, in1=styles)
            styles_h = singles.tile([128, 1], F32)
            nc.vector.tensor_scalar_mul(out=styles_h, in0=styles, scalar1=0.5)

            wnat_t = wnat.rearrange("p (ci t) -> p t ci", t=NTAPS)
            wj = singles.tile([128, 9 * NJ * 128], BF16)
            ps_dm = ps_a.tile([128, 1], F32, tag="ps_dm")

            # (1) G-matrix combos in NATURAL [co,ci] layout on GpSimd straight
            # from wnat: no dependence on PE/V, so they stream immediately.
            combo = {}
            for g in [3, 4, 5, 6, 7, 8, 0, 1, 2]:
                nat = [wnat_t[:, g * 3 + kw, :] for kw in range(3)]
                s01 = tpool.tile([128, 128], F32, tag="s01", name=f"s01_{g}")
                nc.gpsimd.tensor_add(out=s01, in0=nat[0], in1=nat[2])
                u0 = upool.tile([128, 128], F32, tag="u0", name=f"u0_{g}")
                nc.gpsimd.tensor_add(out=u0, in0=s01, in1=nat[1])
                u1 = upool.tile([128, 128], F32, tag="u1", name=f"u1_{g}")
                nc.gpsimd.tensor_sub(out=u1, in0=s01, in1=nat[1])
                combo[g] = (nat, u0, u1)

            # (2) wjv transposes + scale-application from PSUM (V and S).
            for g in [3, 4, 5, 6, 7, 8, 0, 1, 2]:
                nat, u0, u1 = combo[g]
                base = g * NJ * 128
                wjv = [wj[:, base + j * 128: base + (j + 1) * 128] for j in range(4)]
                srcs = [(nat[0], wjv[0], styles), (u0, wjv[1], styles_h),
                        (u1, wjv[2], styles_h), (nat[2], wjv[3], styles)]
                for si_, (src, dst, sc) in enumerate(srcs):
                    ppw = ps_a.tile([128, 128], F32, tag="paw", name=f"pp{g}_{si_}")
                    nc.tensor.transpose(ppw, src, ident)
                    if si_ in (0, 3):
                        nc.vector.tensor_scalar_mul(out=dst, in0=ppw, scalar1=sc)
                    else:
                        nc.scalar.activation(out=dst, in_=ppw, func=AF.Identity,
                                             scale=sc)

            # (3) sumsq squares + matmuls: after the wjv path so the S acts
            # run first; still ahead of the conv in the in-order PE queue.
            for gi, g in enumerate([3, 4, 5, 6, 7, 8, 0, 1, 2]):
                for kw in range(3):
                    t = g * 3 + kw
                    pps = ps_a.tile([128, 128], F32, tag="paw", name=f"pq{t}")
                    nc.tensor.transpose(pps, wnat_t[:, t, :], ident)
                    sq = sqpool.tile([128, 128], F32, tag="sq", name=f"sq{t}")
                    nc.scalar.activation(out=sq, in_=pps, func=AF.Square)
                    nc.tensor.matmul(ps_dm, lhsT=sq, rhs=styles2,
                                     start=(gi == 0 and kw == 0),
                                     stop=(gi == 8 and kw == 2))


            transform_block(0, st0)
            transform_block(1, st1)

            # scale[co] = GAIN * rsqrt(sumsq+EPS)
            eps_sb = singles.tile([128, 1], F32)
            nc.vector.memset(eps_sb, EPS / (GAIN * GAIN))
            sc_tmp = singles.tile([128, 1], F32)
            nc.scalar.activation(out=sc_tmp, in_=ps_dm, func=AF.Sqrt,
                                 bias=eps_sb, scale=1.0 / (GAIN * GAIN))
            scale_sb = singles.tile([128, 1], F32)
            nc.vector.reciprocal(out=scale_sb, in_=sc_tmp)
            bias_g = singles.tile([128, 1], F32)
            nc.scalar.mul(out=bias_g, in_=bs_sb, mul=GAIN)

        pspool = ctx.enter_context(tc.tile_pool(name="pspool", bufs=2, space="PSUM"))

        # ---- phase B: winograd-domain conv, one d-slice per psum group ----
        # Transform work trickles in half-block chunks at the END of each
        # d-iteration so V/G queue bursts stay ~2us and never delay the
        # PSUM-releasing epilogue ops.
        staged = {0: st0, 1: st1, 2: st2}
        for d in range(R):
            # trickle-in staging + transform halves for blocks 2..7, ahead
            # of this d's matmuls so they never delay the epilogue ops
            if d % 4 == 2 and d // 4 + 3 < NBLK:
                staged[d // 4 + 3] = stage_block(d // 4 + 3)
            if d % 2 == 0 and d // 2 < 2 * (NBLK - 2):
                blk, half = 2 + d // 4, (d // 2) % 2
                transform_part(blk, staged[blk], half * 2, half * 2 + 2)

            kds = [kd for kd in range(3) if 0 <= d + kd - 1 < R]
            gs = [kd * 3 + kh for kd in kds for kh in range(3)]
            ps = [pspool.tile([128, R, NT], F32, tag=f"ps{j}", name=f"ps{d}_{j}")
                  for j in range(NJ)]

            for gi, g in enumerate(gs):
                kd, kh = g // 3, g % 3
                s = d + kd - 1
                xt = xblocks[s // DBLK]
                si = s % DBLK
                for j in range(NJ):
                    nc.tensor.matmul(
                        ps[j], lhsT=wj[:, (g * NJ + j) * 128:(g * NJ + j + 1) * 128],
                        rhs=xt[:, si, kh:kh + R, j, :],
                        start=(gi == 0), stop=(gi == len(gs) - 1))

            # inverse transform + fused prelu epilogue + store
            # m1/m2 drained to SBUF by ScalarE; V handles the two remaining
            # PSUM reads (one PSUM operand per TensorTensor), G the SBUF-only.
            ob = obpool.tile([128, R, R], F32, tag="ob", name=f"ob{d}")
            s1 = tpool.tile([128, R, NT], F32, tag="s1", name=f"s1_{d}")
            s2 = tpool.tile([128, R, NT], F32, tag="s2", name=f"s2_{d}")
            t_e = tpool.tile([128, R, NT], F32, tag="t_e", name=f"te{d}")
            t_e2 = tpool.tile([128, R, NT], F32, tag="t_e2", name=f"te2{d}")
            t_o = tpool.tile([128, R, NT], F32, tag="t_o", name=f"to{d}")
            t_o2 = tpool.tile([128, R, NT], F32, tag="t_o2", name=f"to2{d}")
            nc.scalar.activation(out=s1, in_=ps[1], func=AF.Identity)
            nc.scalar.activation(out=s2, in_=ps[2], func=AF.Identity)
            nc.vector.tensor_add(out=t_e, in0=ps[0], in1=s1)
            nc.gpsimd.tensor_add(out=t_e2, in0=t_e, in1=s2)
            nc.gpsimd.tensor_sub(out=t_o, in0=s1, in1=s2)
            nc.vector.tensor_sub(out=t_o2, in0=t_o, in1=ps[3])
            nc.scalar.activation(out=ob[:, :, 0::2], in_=t_e2, func=AF.Prelu,
                                 bias=bias_g, scale=scale_sb, alpha=SLOPE)
            nc.scalar.activation(out=ob[:, :, 1::2], in_=t_o2, func=AF.Prelu,
                                 bias=bias_g, scale=scale_sb, alpha=SLOPE)
            nc.sync.dma_start(out=out_d[:, d * 1024:(d + 1) * 1024],
                              in_=ob.rearrange("p h w -> p (h w)"))

    nc.compile()
    return nc


def kernel(**inputs):
    x = np.ascontiguousarray(np.asarray(inputs["x"], dtype=np.float32))
    w = np.ascontiguousarray(np.asarray(inputs["w"], dtype=np.float32))
    weight = np.ascontiguousarray(np.asarray(inputs["weight"], dtype=np.float32))
    aw = np.ascontiguousarray(np.asarray(inputs["affine_weight"], dtype=np.float32))
    ab = np.ascontiguousarray(np.asarray(inputs["affine_bias"], dtype=np.float32))
    bias = np.ascontiguousarray(np.asarray(inputs["bias"], dtype=np.float32))

    if "nc" not in _cache:
        _cache["nc"] = _build()
    nc = _cache["nc"]

    wt2 = weight.reshape(COUT, CIN * NTAPS)
    in_maps = [
        {
            "x": x[b].reshape(CIN, R * R * R),
            "wvec": w[b],
            "weight": wt2,
            "aw": aw,
            "ab": ab,
            "bias": bias,
        }
        for b in range(B)
    ]
    res = run_bass_kernel_spmd(nc, in_maps, list(range(NCORES)))
    out = np.stack([res.results[b]["out"].reshape(COUT, R, R, R) for b in range(B)])
    return out.astype(np.float32)


def run_traced(**inputs):
    """Like kernel(), but also returns the profiled HW exec time in ns."""
    x = np.asarray(inputs["x"], dtype=np.float32)
    w = np.asarray(inputs["w"], dtype=np.float32)
    weight = np.asarray(inputs["weight"], dtype=np.float32)
    aw = np.asarray(inputs["affine_weight"], dtype=np.float32)
    ab = np.asarray(inputs["affine_bias"], dtype=np.float32)
    bias = np.asarray(inputs["bias"], dtype=np.float32)
    if "nc" not in _cache:
        _cache["nc"] = _build()
    nc = _cache["nc"]
    wt2 = weight.reshape(COUT, CIN * NTAPS)
    in_maps = [
        {"x": x[b].reshape(CIN, R * R * R), "wvec": w[b], "weight": wt2,
         "aw": aw, "ab": ab, "bias": bias}
        for b in range(B)
    ]
    res = run_bass_kernel_spmd(nc, in_maps, list(range(NCORES)), trace=True)
    out = np.stack([res.results[b]["out"].reshape(COUT, R, R, R) for b in range(B)])
    return out.astype(np.float32), res.exec_time_ns, res


# revision 42
# speedup vs baseline: 1.0023x; 1.0023x over previous
"""Trainium2 Bass kernel for nn_Conv3DSynthesisLayer.

Computes, per sample b (one NeuronCore each, data-parallel over batch B=8):
  styles = w[b] @ (affine_weight / sqrt(512)).T + affine_bias        [Cin]
  wmod   = weight * styles[None,:,None..] ; demod by rsqrt(sumsq)    [Cout,Cin,3,3,3]
  out    = lrelu(conv3d(x[b], wmod, pad=1) + bias) * sqrt(2)         [Cout,32,32,32]

v3 (Winograd, ~330us vs 434us direct-bf16 baseline):
  Winograd F(2,3) along the W axis, bf16 (hw rel err 3.1e-3).
  - The kw-dim of the conv runs in the 4-point transform domain: per
    2-output tile t (16 tiles/row, padded coords), X~0 = x[2t-1]-x[2t+1],
    X~1 = x[2t]+x[2t+1], X~2 = x[2t+1]-x[2t], X~3 = x[2t]-x[2t+2];
    W~0 = w0, W~1 = (w0+w1+w2)/2, W~2 = (w0-w1+w2)/2, W~3 = w2.
  - Per output d-slice: 4 psum banks (one per j), each accumulating 9
    (kd,kh)-tap matmuls of full 512 rows (K=Cin=128 on partitions):
    36 matmuls/slice vs 54 for direct conv -> 1.5x less PE work.
    (fp8 e4m3 DoubleRow would be 2x but fails the 2e-2 gate at 3.6e-2;
    int8 and e3m4 perf modes are rejected by walrus/the TRN2 ISA.)
  - Phase A ordering keeps the in-order PE queue clear for an early conv
    start: GpSimd does the G-matrix combos in natural [co,ci] layout
    straight from wnat; the wjv transposes feed V (tensor_scalar_mul) and
    ScalarE (Identity-with-scale) directly from PSUM; the demod-sumsq
    squares+matmuls are emitted after the wjv path.
  - Inverse transform (even w: m0+m1+m2, odd w: m1-m2-m3): ScalarE drains
    m1/m2 to SBUF (TensorTensor allows one PSUM operand; GpSimd cannot
    touch PSUM), V adds with ps0/ps3, G the SBUF-only ops, then the fused
    Prelu epilogue (demod scale + bias + lrelu gain) on ScalarE with
    strided even/odd writes, then DMA per slice.
  - X~ is built from the raw staged x in half-block chunks trickled at the
    top of d-iterations, so V/G queue bursts stay ~2us and never delay the
    PSUM-releasing epilogue ops.
"""
import sys

sys.path.insert(0, "/opt/trn_rl_repo")

import numpy as np
from contextlib import ExitStack

import concourse.mybir as mybir
import concourse.tile as tile
from concourse import bacc
from concourse.masks import make_identity
from concourse.bass_utils import run_bass_kernel_spmd

F32 = mybir.dt.float32
BF16 = mybir.dt.bfloat16
AF = mybir.ActivationFunctionType

B, CIN, COUT, R = 8, 128, 128, 32
W_DIM = 512
NTAPS = 27
HP = R + 2    # 34: padded h extent of X~
NJ = 4        # winograd transform points
NT = 16       # w-tiles per row (2 outputs each)
GAIN = float(np.sqrt(2.0).astype(np.float32))
SLOPE = 0.2
EPS = 1e-8
DBLK = 4      # d-slices per x block
NBLK = R // DBLK
NCORES = 8

_cache = {}


def _build():
    nc = bacc.Bacc("TRN2", target_bir_lowering=False, debug=False, num_devices=NCORES)
    x_d = nc.dram_tensor("x", [CIN, R * R * R], F32, kind="ExternalInput").ap()
    wv_d = nc.dram_tensor("wvec", [W_DIM], F32, kind="ExternalInput").ap()
    wt_d = nc.dram_tensor("weight", [COUT, CIN * NTAPS], F32, kind="ExternalInput").ap()
    aw_d = nc.dram_tensor("aw", [CIN, W_DIM], F32, kind="ExternalInput").ap()
    ab_d = nc.dram_tensor("ab", [CIN], F32, kind="ExternalInput").ap()
    bs_d = nc.dram_tensor("bias", [COUT], F32, kind="ExternalInput").ap()
    out_d = nc.dram_tensor("out", [COUT, R * R * R], F32, kind="ExternalOutput").ap()

    ctx = ExitStack()
    with ctx:
        tc = ctx.enter_context(tile.TileContext(nc))
        singles = ctx.enter_context(tc.tile_pool(name="singles", bufs=1))
        xpool = ctx.enter_context(tc.tile_pool(name="xpool", bufs=4))
        stpool = ctx.enter_context(tc.tile_pool(name="stpool", bufs=3))
        sqpool = ctx.enter_context(tc.tile_pool(name="sqpool", bufs=3))
        upool = ctx.enter_context(tc.tile_pool(name="upool", bufs=9))
        tpool = ctx.enter_context(tc.tile_pool(name="tpool", bufs=2))
        obpool = ctx.enter_context(tc.tile_pool(name="obpool", bufs=3))

        # ---- param DMAs up front ----
        aw_sb = singles.tile([128, W_DIM], F32)
        nc.sync.dma_start(out=aw_sb, in_=aw_d)
        wnat = singles.tile([128, CIN * NTAPS], F32)
        nc.sync.dma_start(out=wnat, in_=wt_d)
        wv_sb = singles.tile([128, 4], F32)
        nc.sync.dma_start(out=wv_sb, in_=wv_d.rearrange("(c k) -> k c", k=128))
        ab_sb = singles.tile([128, 1], F32)
        nc.sync.dma_start(out=ab_sb, in_=ab_d.rearrange("(p one) -> p one", one=1))
        bs_sb = singles.tile([128, 1], F32)
        nc.sync.dma_start(out=bs_sb, in_=bs_d.rearrange("(p one) -> p one", one=1))

        # ---- x staging + winograd input transform ----
        x_r = x_d.rearrange("p (d hw) -> p d hw", hw=R * R)
        xblocks = [None] * NBLK

        def stage_block(blk):
            stag = stpool.tile([128, DBLK, R, R], F32, tag="stag", name=f"st{blk}")
            nc.sync.dma_start(
                out=stag.rearrange("p d h w -> p d (h w)"),
                in_=x_r[:, blk * DBLK:(blk + 1) * DBLK, :])
            return stag

        def transform_part(blk, stag, s0, s1):
            """X~[si, hp(34), j(4), t(16)] bf16 from raw stag, si in [s0,s1)."""
            if xblocks[blk] is None:
                xblocks[blk] = xpool.tile([128, DBLK, HP, NJ, NT], BF16,
                                          tag="xt", name=f"xt{blk}")
            xt = xblocks[blk]
            v = nc.vector
            g = nc.gpsimd
            st = stag[:, s0:s1]
            # h-pad rows 0 and 33 are transforms of zero rows
            v.memset(xt[:, s0:s1, 0, :, :], 0.0)
            g.memset(xt[:, s0:s1, HP - 1, :, :], 0.0)
            hp = xt[:, s0:s1, 1:R + 1, :, :]
            # j0 = x[2t-1] - x[2t+1]: t=0 -> -x[1]; t 1..15
            v.tensor_scalar_mul(out=hp[:, :, :, 0, 0:1],
                                in0=st[:, :, :, 1:2], scalar1=-1.0)
            v.tensor_sub(out=hp[:, :, :, 0, 1:16],
                         in0=st[:, :, :, 1:30:2], in1=st[:, :, :, 3:32:2])
            # j1 = x[2t] + x[2t+1];  j2 = x[2t+1] - x[2t]
            v.tensor_add(out=hp[:, :, :, 1, :],
                         in0=st[:, :, :, 0:32:2], in1=st[:, :, :, 1:32:2])
            g.tensor_sub(out=hp[:, :, :, 2, :],
                         in0=st[:, :, :, 1:32:2], in1=st[:, :, :, 0:32:2])
            # j3 = x[2t] - x[2t+2]: t 0..14; t=15 -> x[30]
            g.tensor_sub(out=hp[:, :, :, 3, 0:15],
                         in0=st[:, :, :, 0:29:2], in1=st[:, :, :, 2:31:2])
            g.tensor_copy(out=hp[:, :, :, 3, 15:16], in_=st[:, :, :, 30:31])

        def transform_block(blk, stag):
            transform_part(blk, stag, 0, DBLK)

        st0 = stage_block(0)
        st1 = stage_block(1)
        st2 = stage_block(2)

        # ---- phase A: styles, winograd-domain bf16 weights, demod scale ----
        with tc.tile_pool(name="ps_a", bufs=2, space="PSUM") as ps_a:
            ident = singles.tile([128, 128], F32)
            make_identity(nc, ident)

            # affine_weight.T, chunked over the 512-dim
            awt = singles.tile([128, W_DIM], F32)
            for c in range(4):
                paw = ps_a.tile([128, 128], F32, tag="paw", name=f"paw{c}")
                nc.tensor.transpose(paw, aw_sb[:, c * 128:(c + 1) * 128], ident)
                nc.vector.tensor_copy(out=awt[:, c * 128:(c + 1) * 128], in_=paw)

            ps_sty = ps_a.tile([128, 1], F32, tag="ps_sty")
            for c in range(4):
                nc.tensor.matmul(ps_sty, lhsT=awt[:, c * 128:(c + 1) * 128],
                                 rhs=wv_sb[:, c:c + 1], start=(c == 0), stop=(c == 3))
            styles = singles.tile([128, 1], F32)
            nc.scalar.activation(out=styles, in_=ps_sty, func=AF.Identity,
                                 bias=ab_sb, scale=1.0 / float(np.sqrt(W_DIM)))
            styles2 = singles.tile([128, 1], F32)
            nc.vector.tensor_mul(out=styles2, in0=styles, in1=styles)
            styles_h = singles.tile([128, 1], F32)
            nc.vector.tensor_scalar_mul(out=styles_h, in0=styles, scalar1=0.5)

            wnat_t = wnat.rearrange("p (ci t) -> p t ci", t=NTAPS)
            wj = singles.tile([128, 9 * NJ * 128], BF16)
            ps_dm = ps_a.tile([128, 1], F32, tag="ps_dm")

            # (1) G-matrix combos in NATURAL [co,ci] layout on GpSimd straight
            # from wnat: no dependence on PE/V, so they stream immediately.
            combo = {}
            for g in [3, 4, 5, 6, 7, 8, 0, 1, 2]:
                nat = [wnat_t[:, g * 3 + kw, :] for kw in range(3)]
                s01 = tpool.tile([128, 128], F32, tag="s01", name=f"s01_{g}")
                nc.gpsimd.tensor_add(out=s01, in0=nat[0], in1=nat[2])
                u0 = upool.tile([128, 128], F32, tag="u0", name=f"u0_{g}")
                nc.gpsimd.tensor_add(out=u0, in0=s01, in1=nat[1])
                u1 = upool.tile([128, 128], F32, tag="u1", name=f"u1_{g}")
                nc.gpsimd.tensor_sub(out=u1, in0=s01, in1=nat[1])
                combo[g] = (nat, u0, u1)

            # (2) wjv transposes + scale-application from PSUM (V and S).
            for g in [3, 4, 5, 6, 7, 8, 0, 1, 2]:
                nat, u0, u1 = combo[g]
                base = g * NJ * 128
                wjv = [wj[:, base + j * 128: base + (j + 1) * 128] for j in range(4)]
                srcs = [(nat[0], wjv[0], styles), (u0, wjv[1], styles_h),
                        (u1, wjv[2], styles_h), (nat[2], wjv[3], styles)]
                for si_, (src, dst, sc) in enumerate(srcs):
                    ppw = ps_a.tile([128, 128], F32, tag="paw", name=f"pp{g}_{si_}")
                    nc.tensor.transpose(ppw, src, ident)
                    if si_ in (0, 3):
                        nc.vector.tensor_scalar_mul(out=dst, in0=ppw, scalar1=sc)
                    else:
                        nc.scalar.activation(out=dst, in_=ppw, func=AF.Identity,
                                             scale=sc)

            # (3) sumsq squares + matmuls: after the wjv path so the S acts
            # run first; still ahead of the conv in the in-order PE queue.
            for gi, g in enumerate([3, 4, 5, 6, 7, 8, 0, 1, 2]):
                for kw in range(3):
                    t = g * 3 + kw
                    pps = ps_a.tile([128, 128], F32, tag="paw", name=f"pq{t}")
                    nc.tensor.transpose(pps, wnat_t[:, t, :], ident)
                    sq = sqpool.tile([128, 128], F32, tag="sq", name=f"sq{t}")
                    nc.scalar.activation(out=sq, in_=pps, func=AF.Square)
                    nc.tensor.matmul(ps_dm, lhsT=sq, rhs=styles2,
                                     start=(gi == 0 and kw == 0),
                                     stop=(gi == 8 and kw == 2))


            transform_block(0, st0)
            transform_block(1, st1)

            # scale[co] = GAIN * rsqrt(sumsq+EPS)
            eps_sb = singles.tile([128, 1], F32)
            nc.vector.memset(eps_sb, EPS / (GAIN * GAIN))
            sc_tmp = singles.tile([128, 1], F32)
            nc.scalar.activation(out=sc_tmp, in_=ps_dm, func=AF.Sqrt,
                                 bias=eps_sb, scale=1.0 / (GAIN * GAIN))
            scale_sb = singles.tile([128, 1], F32)
            nc.vector.reciprocal(out=scale_sb, in_=sc_tmp)
            bias_g = singles.tile([128, 1], F32)
            nc.scalar.mul(out=bias_g, in_=bs_sb, mul=GAIN)

        pspool = ctx.enter_context(tc.tile_pool(name="pspool", bufs=2, space="PSUM"))

        # ---- phase B: winograd-domain conv, one d-slice per psum group ----
        # Transform work trickles in half-block chunks at the END of each
        # d-iteration so V/G queue bursts stay ~2us and never delay the
        # PSUM-releasing epilogue ops.
        staged = {0: st0, 1: st1, 2: st2}
        for d in range(R):
            # trickle-in staging + transform halves for blocks 2..7, ahead
            # of this d's matmuls so they never delay the epilogue ops
            if d % 4 == 2 and d // 4 + 3 < NBLK:
                staged[d // 4 + 3] = stage_block(d // 4 + 3)
            if d % 2 == 0 and d // 2 < 2 * (NBLK - 2):
                blk, half = 2 + d // 4, (d // 2) % 2
                transform_part(blk, staged[blk], half * 2, half * 2 + 2)

            kds = [kd for kd in range(3) if 0 <= d + kd - 1 < R]
            gs = [kd * 3 + kh for kd in kds for kh in range(3)]
            ps = [pspool.tile([128, R, NT], F32, tag=f"ps{j}", name=f"ps{d}_{j}")
                  for j in range(NJ)]

            for gi, g in enumerate(gs):
                kd, kh = g // 3, g % 3
                s = d + kd - 1
                xt = xblocks[s // DBLK]
                si = s % DBLK
                for j in range(NJ):
                    nc.tensor.matmul(
                        ps[j], lhsT=wj[:, (g * NJ + j) * 128:(g * NJ + j + 1) * 128],
                        rhs=xt[:, si, kh:kh + R, j, :],
                        start=(gi == 0), stop=(gi == len(gs) - 1))

            # inverse transform + fused prelu epilogue + store
            # m1/m2 drained to SBUF by ScalarE; V handles the two remaining
            # PSUM reads (one PSUM operand per TensorTensor), G the SBUF-only.
            ob = obpool.tile([128, R, R], F32, tag="ob", name=f"ob{d}")
            s1 = tpool.tile([128, R, NT], F32, tag="s1", name=f"s1_{d}")
            s2 = tpool.tile([128, R, NT], F32, tag="s2", name=f"s2_{d}")
            t_e = tpool.tile([128, R, NT], F32, tag="t_e", name=f"te{d}")
            t_e2 = tpool.tile([128, R, NT], F32, tag="t_e2", name=f"te2{d}")
            t_o = tpool.tile([128, R, NT], F32, tag="t_o", name=f"to{d}")
            t_o2 = tpool.tile([128, R, NT], F32, tag="t_o2", name=f"to2{d}")
            nc.scalar.activation(out=s1, in_=ps[1], func=AF.Identity)
            nc.scalar.activation(out=s2, in_=ps[2], func=AF.Identity)
            nc.vector.tensor_add(out=t_e, in0=ps[0], in1=s1)
            nc.gpsimd.tensor_add(out=t_e2, in0=t_e, in1=s2)
            nc.gpsimd.tensor_sub(out=t_o, in0=s1, in1=s2)
            nc.vector.tensor_sub(out=t_o2, in0=t_o, in1=ps[3])
            nc.scalar.activation(out=ob[:, :, 0::2], in_=t_e2, func=AF.Prelu,
                                 bias=bias_g, scale=scale_sb, alpha=SLOPE)
            nc.scalar.activation(out=ob[:, :, 1::2], in_=t_o2, func=AF.Prelu,
                                 bias=bias_g, scale=scale_sb, alpha=SLOPE)
            nc.sync.dma_start(out=out_d[:, d * 1024:(d + 1) * 1024],
                              in_=ob.rearrange("p h w -> p (h w)"))

    nc.compile()
    return nc


def kernel(**inputs):
    x = np.ascontiguousarray(np.asarray(inputs["x"], dtype=np.float32))
    w = np.ascontiguousarray(np.asarray(inputs["w"], dtype=np.float32))
    weight = np.ascontiguousarray(np.asarray(inputs["weight"], dtype=np.float32))
    aw = np.ascontiguousarray(np.asarray(inputs["affine_weight"], dtype=np.float32))
    ab = np.ascontiguousarray(np.asarray(inputs["affine_bias"], dtype=np.float32))
    bias = np.ascontiguousarray(np.asarray(inputs["bias"], dtype=np.float32))

    if "nc" not in _cache:
        _cache["nc"] = _build()
    nc = _cache["nc"]

    wt2 = weight.reshape(COUT, CIN * NTAPS)
    in_maps = [
        {
            "x": x[b].reshape(CIN, R * R * R),
            "wvec": w[b],
            "weight": wt2,
            "aw": aw,
            "ab": ab,
            "bias": bias,
        }
        for b in range(B)
    ]
    res = run_bass_kernel_spmd(nc, in_maps, list(range(NCORES)))
    out = np.stack([res.results[b]["out"].reshape(COUT, R, R, R) for b in range(B)])
    return out.astype(np.float32)


def run_traced(**inputs):
    """Like kernel(), but also returns the profiled HW exec time in ns."""
    x = np.asarray(inputs["x"], dtype=np.float32)
    w = np.asarray(inputs["w"], dtype=np.float32)
    weight = np.asarray(inputs["weight"], dtype=np.float32)
    aw = np.asarray(inputs["affine_weight"], dtype=np.float32)
    ab = np.asarray(inputs["affine_bias"], dtype=np.float32)
    bias = np.asarray(inputs["bias"], dtype=np.float32)
    if "nc" not in _cache:
        _cache["nc"] = _build()
    nc = _cache["nc"]
    wt2 = weight.reshape(COUT, CIN * NTAPS)
    in_maps = [
        {"x": x[b].reshape(CIN, R * R * R), "wvec": w[b], "weight": wt2,
         "aw": aw, "ab": ab, "bias": bias}
        for b in range(B)
    ]
    res = run_bass_kernel_spmd(nc, in_maps, list(range(NCORES)), trace=True)
    out = np.stack([res.results[b]["out"].reshape(COUT, R, R, R) for b in range(B)])
    return out.astype(np.float32), res.exec_time_ns, res
